# revision 36
# baseline (speedup 1.0000x reference)
"""Trainium2 Bass kernel for nn_MultiHeadModel (moe_routing).

Reference computes a dense MoE: every one of E=8 organ heads over the full
batch B=4096, then selects each sample's routed head.  Only the routed
compute is needed (1/8 of the dense FLOPs).  Strategy:

  host:   group samples by organ_idx, pad each group to a common capacity C
  device: expert-parallel SPMD -- core e runs the two-layer MLPs (immune +
          gene heads) of expert e over its padded group, fp16 inputs with
          fp32 PSUM accumulation
  host:   scatter per-expert results back to batch order

Per core:  hTg = relu(W1g.T @ fusedT + b1g)   [H, C]
           gene = hTg.T @ W2_gene             [C, G]
           hTi = relu(W1i.T @ fusedT + b1i)   [H, C]
           imm  = w2_imm.T @ hTi              [1, C]
b2 biases are added on host after the scatter (mathematically identical).

Phases are ordered gene-layer1 -> gene-layer2 -> imm-layer1 -> imm-dot so
the input DMA streams (fusedT+W1g, then W2g, then W1i) land just ahead of
the phase that consumes them; layer 1 runs k-outer over small groups of
output chunks so the PE has work as soon as each 128-row slab lands and the
HAM clock stays warm through the DMA-bound start.
"""

import os
import sys

import numpy as np

for _p in ("/opt/trn_rl_repo", os.path.expanduser("~/.axon_site/_ro/trn_rl_repo")):
    if os.path.isdir(_p) and _p not in sys.path:
        sys.path.insert(0, _p)

import concourse.tile as tile
from concourse import bacc, mybir
from concourse.bass_utils import run_bass_kernel_spmd
from concourse.tile_rust import add_dep_helper

B, DV, DT, H, G, E = 4096, 1024, 1024, 1024, 4096, 8
D = DV + DT
P = 128
KD = D // P   # 16 contraction chunks for layer 1
KH = H // P   # 8 contraction chunks for layer 2
NG = G // 512  # 8 free-dim chunks over G

_cache: dict[int, object] = {}


def _build(C: int):
    """Build + finalize the per-core Bass program for group capacity C."""
    f16, f32 = mybir.dt.float16, mybir.dt.float32
    nc = bacc.Bacc(None)

    fusedT_p = nc.declare_dram_parameter("fusedT", [D, C], f16, False)
    w1i_p = nc.declare_dram_parameter("w1i", [KD, P, H], f16, False)
    w1g_p = nc.declare_dram_parameter("w1g", [KD, P, H], f16, False)
    w2g_p = nc.declare_dram_parameter("w2g", [KH, P, G], f16, False)
    b1i_p = nc.declare_dram_parameter("b1i", [P, KH], f32, False)
    b1g_p = nc.declare_dram_parameter("b1g", [P, KH], f32, False)
    w2i_p = nc.declare_dram_parameter("w2i", [P, KH], f16, False)
    gene_p = nc.declare_dram_parameter("gene_out", [C, G], f32, True)
    imm_p = nc.declare_dram_parameter("imm_out", [1, C], f32, True)

    # free-dim chunks over C (<=512 each)
    nch = [(o, min(512, C - o)) for o in range(0, C, 512)]
    # partition chunks over C for layer 2
    mch = [(o, min(P, C - o)) for o in range(0, C, P)]

    Relu = mybir.ActivationFunctionType.Relu

    with tile.TileContext(nc) as tc:
        with (
            tc.tile_pool(name="res", bufs=1) as res,
            tc.tile_pool(name="gout", bufs=4) as gout,
            tc.tile_pool(name="ps", bufs=8, space="PSUM") as ps,
        ):
            fT = res.tile([P, KD, C], f16, tag="fT")
            w1gs = res.tile([P, KD, H], f16, tag="w1g")
            w1is = res.tile([P, KD, H], f16, tag="w1i")
            w2gs = res.tile([P, KH, G], f16, tag="w2g")
            hTi = res.tile([P, KH, C], f16, tag="hTi")
            hTg = res.tile([P, KH, C], f16, tag="hTg")
            b1is = res.tile([P, KH], f32, tag="b1i")
            b1gs = res.tile([P, KH], f32, tag="b1g")
            w2is = res.tile([P, KH], f16, tag="w2i")
            imms = res.tile([1, C], f32, tag="imms")
            scr = res.tile([P, 384], f16, tag="scr")  # warm-up garbage

            # Input loads in phase-consumption order, interleaved across the
            # sync and gpsimd queues: fT+w1g (phase 1), w2g (phase 2),
            # w1i (phase 3).
            for k in range(KD):
                ea, eb = (nc.sync, nc.gpsimd) if k % 2 == 0 else (
                    nc.gpsimd,
                    nc.sync,
                )
                ea.dma_start(fT[:, k, :], fusedT_p[k * P : (k + 1) * P, :])
                eb.dma_start(w1gs[:, k, :], w1g_p[k])
            nc.sync.dma_start(b1gs[:], b1g_p[:])
            nc.sync.dma_start(b1is[:], b1i_p[:])
            nc.sync.dma_start(w2is[:], w2i_p[:])
            def layer1(w1s, b1s, hT, hook_k=KD // 2):
                hook_mm = None
                # column-chunk outer, k-outer over all KH output chunks:
                # every arriving 128-row slab immediately enables KH matmuls,
                # so the PE saturates against the input DMA stream.
                for ci, (ns, nz) in enumerate(nch):
                    pts = [
                        ps.tile([P, 512], f32, tag="ps", name=f"ps_{m}")[:, :nz]
                        for m in range(KH)
                    ]
                    for k in range(KD):
                        for m in range(KH):
                            mi = nc.tensor.matmul(
                                pts[m],
                                w1s[:, k, m * P : (m + 1) * P],
                                fT[:, k, ns : ns + nz],
                                start=(k == 0),
                                stop=(k == KD - 1),
                            )
                        if ci == 0 and k == hook_k:
                            hook_mm = mi
                    for m in range(KH):
                        if m % 2:
                            nc.vector.tensor_scalar(
                                hT[:, m, ns : ns + nz],
                                pts[m],
                                b1s[:, m : m + 1],
                                0.0,
                                mybir.AluOpType.add,
                                mybir.AluOpType.max,
                            )
                        else:
                            nc.scalar.activation(
                                hT[:, m, ns : ns + nz],
                                pts[m],
                                Relu,
                                bias=b1s[:, m : m + 1],
                                scale=1.0,
                            )
                return hook_mm

            # phase 0: PE warm-up on garbage during the dead DMA-start window
            # (real inputs land ~12us in; HAM needs ~3.4us of sustained PE
            # activity to lift the clock from 1.2 to 2.4 GHz)
            wpt = ps.tile([P, 512], f32, tag="ps", name="ps_warm")
            nc.vector.memset(scr[:], 0.0)
            # force the ACT function-table load off the critical path
            nc.scalar.activation(scr[:1, :1], scr[:1, 256:257], Relu)
            for _ in range(30):
                nc.tensor.matmul(
                    wpt[:, :256], scr[:, :128], scr[:, 128:384], start=True,
                    stop=True, skip_group_check=True,
                )

            # phase 1: gene head layer 1
            g_mid = layer1(w1gs, b1gs, hTg)

            # w1i loads start mid-phase-1 (dep gate) but are emitted after it
            # so the phase's DMA-completion wait thresholds stay unaffected;
            # this keeps the w1i stream from stealing HBM bandwidth from the
            # fT/w1g stream the PE is actually waiting on.
            for k in range(KD):
                dd = (nc.sync if k % 2 else nc.gpsimd).dma_start(
                    w1is[:, k, :], w1i_p[k]
                )
                add_dep_helper(g_mid.ins, dd.ins, reason="w1i after fT/w1g")

            # phase 2: immune head layer 1
            i_start = layer1(w1is, b1is, hTi, hook_k=0)

            # w2g loads start when phase 2 starts, emitted after it
            for k in range(KH):
                dd = (nc.sync if k % 2 else nc.gpsimd).dma_start(
                    w2gs[:, k, :], w2g_p[k]
                )
                add_dep_helper(i_start.ins, dd.ins, reason="w2g after w1i")

            # phase 3: gene head layer 2: gene[m, n] = hTg[:, m].T @ W2g[:, n]
            for n in range(NG):
                for i, (ms, mp) in enumerate(mch):
                    pt = ps.tile([P, 512], f32, tag="ps", name="ps_g")[:mp, :]
                    for k in range(KH):
                        nc.tensor.matmul(
                            pt,
                            hTg[:, k, ms : ms + mp],
                            w2gs[:, k, n * 512 : (n + 1) * 512],
                            start=(k == 0),
                            stop=(k == KH - 1),
                        )
                    ot = gout.tile([P, 512], f32, tag="ot", name="ot")[:mp, :]
                    if i % 2:
                        nc.scalar.copy(out=ot, in_=pt)
                    else:
                        nc.vector.tensor_copy(out=ot, in_=pt)
                    (nc.gpsimd if n % 2 else nc.sync).dma_start(
                        gene_p[ms : ms + mp, n * 512 : (n + 1) * 512], ot
                    )

            # phase 4: immune dot: imm[1, C] = w2i.T @ hTi
            for ns, nz in nch:
                pt = ps.tile([P, 512], f32, tag="ps", name="ps_imm")[:1, :nz]
                for k in range(KH):
                    nc.tensor.matmul(
                        pt,
                        w2is[:, k : k + 1],
                        hTi[:, k, ns : ns + nz],
                        start=(k == 0),
                        stop=(k == KH - 1),
                    )
                nc.vector.tensor_copy(out=imms[:, ns : ns + nz], in_=pt)
            nc.sync.dma_start(imm_p[:], imms[:])

    nc.finalize()
    return nc


def _prep_inputs(inputs):
    fused = np.concatenate(
        [
            np.asarray(inputs["vision_cls"], np.float32),
            np.asarray(inputs["text_cls"], np.float32),
        ],
        axis=1,
    )
    organ = np.asarray(inputs["organ_idx"], np.int32)
    idxs = [np.nonzero(organ == e)[0] for e in range(E)]
    C = max(64, -(-max(len(ix) for ix in idxs) // 64) * 64)

    fused16 = fused.astype(np.float16)
    in_maps = []
    for e in range(E):
        ix = idxs[e]
        ft = np.zeros((D, C), np.float16)
        if len(ix):
            ft[:, : len(ix)] = fused16[ix].T
        in_maps.append(
            {
                "fusedT": ft,
                "w1i": np.asarray(inputs["W1_imm"][e], np.float16).reshape(KD, P, H),
                "w1g": np.asarray(inputs["W1_gene"][e], np.float16).reshape(KD, P, H),
                "w2g": np.asarray(inputs["W2_gene"][e], np.float16).reshape(KH, P, G),
                "b1i": np.ascontiguousarray(
                    np.asarray(inputs["b1_imm"][e], np.float32).reshape(KH, P).T
                ),
                "b1g": np.ascontiguousarray(
                    np.asarray(inputs["b1_gene"][e], np.float32).reshape(KH, P).T
                ),
                "w2i": np.ascontiguousarray(
                    np.asarray(inputs["W2_imm"][e, :, 0], np.float16).reshape(KH, P).T
                ),
            }
        )
    return in_maps, idxs, C


def _run(inputs, trace=False):
    in_maps, idxs, C = _prep_inputs(inputs)
    nc = _cache.get(C)
    if nc is None:
        nc = _build(C)
        _cache[C] = nc

    res = run_bass_kernel_spmd(nc, in_maps, list(range(E)), trace=trace)

    b2_imm = np.asarray(inputs["b2_imm"], np.float32)
    b2_gene = np.asarray(inputs["b2_gene"], np.float32)
    nb = len(np.asarray(inputs["organ_idx"]))
    immune = np.zeros((nb,), np.float32)
    gene = np.zeros((nb, G), np.float32)
    for e in range(E):
        ix = idxs[e]
        n = len(ix)
        if n == 0:
            continue
        imm_e = res.results[e]["imm_out"][0, :n]
        gene_e = res.results[e]["gene_out"][:n]
        if b2_imm[e].any():
            imm_e = imm_e + b2_imm[e, 0]
        if b2_gene[e].any():
            gene_e = gene_e + b2_gene[e][None, :]
        immune[ix] = imm_e
        gene[ix] = gene_e
    vision = np.asarray(inputs["vision_cls"], np.float32)
    text = np.asarray(inputs["text_cls"], np.float32)
    return (immune, gene, vision, text), res


def kernel(**inputs):
    out, _ = _run(inputs, trace=False)
    return out


# revision 38
# speedup vs baseline: 1.1726x; 1.1726x over previous
"""Trainium2 Bass kernel for nn_MultiHeadModel (moe_routing).

Reference computes a dense MoE: every one of E=8 organ heads over the full
batch B=4096, then selects each sample's routed head.  Only the routed
compute is needed (1/8 of the dense FLOPs).  Strategy:

  host:   group samples by organ_idx, pad each group to a common capacity C
  device: expert-parallel SPMD -- core e runs the two-layer MLPs (immune +
          gene heads) of expert e over its padded group, fp16 inputs with
          fp32 PSUM accumulation
  host:   scatter per-expert results back to batch order

Per core:  hTg = relu(W1g.T @ fusedT + b1g)   [H, C]
           gene = hTg.T @ W2_gene             [C, G]
           hTi = relu(W1i.T @ fusedT + b1i)   [H, C]
           imm  = w2_imm.T @ hTi              [1, C]
b2 biases are added on host after the scatter (mathematically identical).

Phases are ordered gene-layer1 -> gene-layer2 -> imm-layer1 -> imm-dot so
the input DMA streams (fusedT+W1g, then W2g, then W1i) land just ahead of
the phase that consumes them; layer 1 runs k-outer over small groups of
output chunks so the PE has work as soon as each 128-row slab lands and the
HAM clock stays warm through the DMA-bound start.
"""

import os
import sys

import numpy as np

for _p in ("/opt/trn_rl_repo", os.path.expanduser("~/.axon_site/_ro/trn_rl_repo")):
    if os.path.isdir(_p) and _p not in sys.path:
        sys.path.insert(0, _p)

import concourse.tile as tile
from concourse import bacc, mybir
from concourse.bass_utils import run_bass_kernel_spmd
from concourse.tile_rust import add_dep_helper

B, DV, DT, H, G, E = 4096, 1024, 1024, 1024, 4096, 8
D = DV + DT
P = 128
KD = D // P   # 16 contraction chunks for layer 1
KH = H // P   # 8 contraction chunks for layer 2
NG = G // 512  # 8 free-dim chunks over G

_cache: dict[int, object] = {}


def _build(C: int):
    """Build + finalize the per-core Bass program for group capacity C."""
    f16, f32 = mybir.dt.float16, mybir.dt.float32
    nc = bacc.Bacc(None)

    fusedT_p = nc.declare_dram_parameter("fusedT", [D, C], f16, False)
    w1i_p = nc.declare_dram_parameter("w1i", [KD, P, H], f16, False)
    w1g_p = nc.declare_dram_parameter("w1g", [KD, P, H], f16, False)
    w2g_p = nc.declare_dram_parameter("w2g", [KH, P, G], f16, False)
    b1i_p = nc.declare_dram_parameter("b1i", [P, KH], f32, False)
    b1g_p = nc.declare_dram_parameter("b1g", [P, KH], f32, False)
    w2i_p = nc.declare_dram_parameter("w2i", [P, KH], f16, False)
    gene_p = nc.declare_dram_parameter("gene_out", [C, G], f32, True)
    imm_p = nc.declare_dram_parameter("imm_out", [1, C], f32, True)

    # free-dim chunks over C (<=512 each)
    nch = [(o, min(512, C - o)) for o in range(0, C, 512)]
    # partition chunks over C for layer 2
    mch = [(o, min(P, C - o)) for o in range(0, C, P)]

    Relu = mybir.ActivationFunctionType.Relu

    with tile.TileContext(nc) as tc:
        with (
            tc.tile_pool(name="res", bufs=1) as res,
            tc.tile_pool(name="gout", bufs=4) as gout,
            tc.tile_pool(name="ps", bufs=8, space="PSUM") as ps,
        ):
            fT = res.tile([P, KD, C], f16, tag="fT")
            w1gs = res.tile([P, KD, H], f16, tag="w1g")
            w1is = res.tile([P, KD, H], f16, tag="w1i")
            w2gs = res.tile([P, KH, G], f16, tag="w2g")
            hTi = res.tile([P, KH, C], f16, tag="hTi")
            hTg = res.tile([P, KH, C], f16, tag="hTg")
            b1is = res.tile([P, KH], f32, tag="b1i")
            b1gs = res.tile([P, KH], f32, tag="b1g")
            w2is = res.tile([P, KH], f16, tag="w2i")
            imms = res.tile([1, C], f32, tag="imms")
            scr = res.tile([P, 384], f16, tag="scr")  # warm-up garbage

            # Input loads in phase-consumption order, interleaved across the
            # sync and gpsimd queues: fT+w1g (phase 1), w2g (phase 2),
            # w1i (phase 3).
            for k in range(KD):
                ea, eb = (nc.sync, nc.gpsimd) if k % 2 == 0 else (
                    nc.gpsimd,
                    nc.sync,
                )
                ea.dma_start(fT[:, k, :], fusedT_p[k * P : (k + 1) * P, :])
                eb.dma_start(w1gs[:, k, :], w1g_p[k])
            nc.sync.dma_start(b1gs[:], b1g_p[:])
            nc.sync.dma_start(b1is[:], b1i_p[:])
            nc.sync.dma_start(w2is[:], w2i_p[:])
            def layer1(w1s, b1s, hT, hook_k=KD // 2):
                hook_mm = None
                # column-chunk outer, k-outer over all KH output chunks:
                # every arriving 128-row slab immediately enables KH matmuls,
                # so the PE saturates against the input DMA stream.
                for ci, (ns, nz) in enumerate(nch):
                    pts = [
                        ps.tile([P, 512], f32, tag="ps", name=f"ps_{m}")[:, :nz]
                        for m in range(KH)
                    ]
                    for k in range(KD):
                        for m in range(KH):
                            mi = nc.tensor.matmul(
                                pts[m],
                                w1s[:, k, m * P : (m + 1) * P],
                                fT[:, k, ns : ns + nz],
                                start=(k == 0),
                                stop=(k == KD - 1),
                            )
                        if ci == 0 and k == hook_k:
                            hook_mm = mi
                    for m in range(KH):
                        if m % 2:
                            nc.vector.tensor_scalar(
                                hT[:, m, ns : ns + nz],
                                pts[m],
                                b1s[:, m : m + 1],
                                0.0,
                                mybir.AluOpType.add,
                                mybir.AluOpType.max,
                            )
                        else:
                            nc.scalar.activation(
                                hT[:, m, ns : ns + nz],
                                pts[m],
                                Relu,
                                bias=b1s[:, m : m + 1],
                                scale=1.0,
                            )
                return hook_mm

            # phase 0: PE warm-up on garbage during the dead DMA-start window
            # (real inputs land ~12us in; HAM needs ~3.4us of sustained PE
            # activity to lift the clock from 1.2 to 2.4 GHz)
            wpt = ps.tile([P, 512], f32, tag="ps", name="ps_warm")
            nc.vector.memset(scr[:], 0.0)
            # force the ACT function-table load off the critical path
            nc.scalar.activation(scr[:1, :1], scr[:1, 256:257], Relu)
            for _ in range(30):
                nc.tensor.matmul(
                    wpt[:, :256], scr[:, :128], scr[:, 128:384], start=True,
                    stop=True, skip_group_check=True,
                )

            # phase 1: gene head layer 1
            g_mid = layer1(w1gs, b1gs, hTg)

            # w1i loads start mid-phase-1 (dep gate) but are emitted after it
            # so the phase's DMA-completion wait thresholds stay unaffected;
            # this keeps the w1i stream from stealing HBM bandwidth from the
            # fT/w1g stream the PE is actually waiting on.
            for k in range(KD):
                dd = (nc.sync if k % 2 else nc.gpsimd).dma_start(
                    w1is[:, k, :], w1i_p[k]
                )
                add_dep_helper(dd.ins, g_mid.ins, reason="w1i after fT/w1g")

            # phase 2: immune head layer 1
            i_start = layer1(w1is, b1is, hTi, hook_k=0)

            # w2g loads start when phase 2 starts, emitted after it
            for k in range(KH):
                dd = (nc.sync if k % 2 else nc.gpsimd).dma_start(
                    w2gs[:, k, :], w2g_p[k]
                )
                add_dep_helper(dd.ins, i_start.ins, reason="w2g after w1i")

            # phase 3: gene head layer 2: gene[m, n] = hTg[:, m].T @ W2g[:, n]
            for n in range(NG):
                for i, (ms, mp) in enumerate(mch):
                    pt = ps.tile([P, 512], f32, tag="ps", name="ps_g")[:mp, :]
                    for k in range(KH):
                        nc.tensor.matmul(
                            pt,
                            hTg[:, k, ms : ms + mp],
                            w2gs[:, k, n * 512 : (n + 1) * 512],
                            start=(k == 0),
                            stop=(k == KH - 1),
                        )
                    ot = gout.tile([P, 512], f32, tag="ot", name="ot")[:mp, :]
                    if i % 2:
                        nc.scalar.copy(out=ot, in_=pt)
                    else:
                        nc.vector.tensor_copy(out=ot, in_=pt)
                    (nc.gpsimd if n % 2 else nc.sync).dma_start(
                        gene_p[ms : ms + mp, n * 512 : (n + 1) * 512], ot
                    )

            # phase 4: immune dot: imm[1, C] = w2i.T @ hTi
            for ns, nz in nch:
                pt = ps.tile([P, 512], f32, tag="ps", name="ps_imm")[:1, :nz]
                for k in range(KH):
                    nc.tensor.matmul(
                        pt,
                        w2is[:, k : k + 1],
                        hTi[:, k, ns : ns + nz],
                        start=(k == 0),
                        stop=(k == KH - 1),
                    )
                nc.vector.tensor_copy(out=imms[:, ns : ns + nz], in_=pt)
            nc.sync.dma_start(imm_p[:], imms[:])

    nc.finalize()
    return nc


def _prep_inputs(inputs):
    fused = np.concatenate(
        [
            np.asarray(inputs["vision_cls"], np.float32),
            np.asarray(inputs["text_cls"], np.float32),
        ],
        axis=1,
    )
    organ = np.asarray(inputs["organ_idx"], np.int32)
    idxs = [np.nonzero(organ == e)[0] for e in range(E)]
    C = max(64, -(-max(len(ix) for ix in idxs) // 64) * 64)

    fused16 = fused.astype(np.float16)
    in_maps = []
    for e in range(E):
        ix = idxs[e]
        ft = np.zeros((D, C), np.float16)
        if len(ix):
            ft[:, : len(ix)] = fused16[ix].T
        in_maps.append(
            {
                "fusedT": ft,
                "w1i": np.asarray(inputs["W1_imm"][e], np.float16).reshape(KD, P, H),
                "w1g": np.asarray(inputs["W1_gene"][e], np.float16).reshape(KD, P, H),
                "w2g": np.asarray(inputs["W2_gene"][e], np.float16).reshape(KH, P, G),
                "b1i": np.ascontiguousarray(
                    np.asarray(inputs["b1_imm"][e], np.float32).reshape(KH, P).T
                ),
                "b1g": np.ascontiguousarray(
                    np.asarray(inputs["b1_gene"][e], np.float32).reshape(KH, P).T
                ),
                "w2i": np.ascontiguousarray(
                    np.asarray(inputs["W2_imm"][e, :, 0], np.float16).reshape(KH, P).T
                ),
            }
        )
    return in_maps, idxs, C


def _run(inputs, trace=False):
    in_maps, idxs, C = _prep_inputs(inputs)
    nc = _cache.get(C)
    if nc is None:
        nc = _build(C)
        _cache[C] = nc

    res = run_bass_kernel_spmd(nc, in_maps, list(range(E)), trace=trace)

    b2_imm = np.asarray(inputs["b2_imm"], np.float32)
    b2_gene = np.asarray(inputs["b2_gene"], np.float32)
    nb = len(np.asarray(inputs["organ_idx"]))
    immune = np.zeros((nb,), np.float32)
    gene = np.zeros((nb, G), np.float32)
    for e in range(E):
        ix = idxs[e]
        n = len(ix)
        if n == 0:
            continue
        imm_e = res.results[e]["imm_out"][0, :n]
        gene_e = res.results[e]["gene_out"][:n]
        if b2_imm[e].any():
            imm_e = imm_e + b2_imm[e, 0]
        if b2_gene[e].any():
            gene_e = gene_e + b2_gene[e][None, :]
        immune[ix] = imm_e
        gene[ix] = gene_e
    vision = np.asarray(inputs["vision_cls"], np.float32)
    text = np.asarray(inputs["text_cls"], np.float32)
    return (immune, gene, vision, text), res


def kernel(**inputs):
    out, _ = _run(inputs, trace=False)
    return out
